# revision 1
# baseline (speedup 1.0000x reference)
import numpy as np
import jax
import jax.numpy as jnp

# nn_DualAttenion: B=32, n=32, H=64, C=256, d_ff=512, dp_rank=8, heads=8
# Sharding: data-parallel over B across the 8 NeuronCores (4 batches/core),
# weights replicated; no cross-core communication needed.

EPS = 1e-5
NUM_HEADS = 8
N_CORES = 8

_WEIGHT_KEYS = (
    'ema_matrix', 'qkv_w', 'qkv_b', 'dpk_w', 'dpk_b', 'dpv_w', 'dpv_b',
    'bn1_g', 'bn1_b', 'bn1_m', 'bn1_v', 'bn2_g', 'bn2_b', 'bn2_m', 'bn2_v',
    'bn3_g', 'bn3_b', 'bn3_m', 'bn3_v',
    'ff1_w1', 'ff1_b1', 'ff1_w2', 'ff1_b2', 'ff2_w1', 'ff2_b1', 'ff2_w2', 'ff2_b2',
)


def _bn(x, g, b, m, v):
    return (x - m) / jnp.sqrt(v + EPS) * g + b


def _ema(x, E):
    L = x.shape[-2]
    return jnp.einsum('bnhad,ga->bnhgd', x, E[:L, :L])


def _dyn_proj(x, w, b):
    p = jax.nn.softmax(x @ w.T + b, axis=-1)
    return jnp.einsum('bnhef,bnhec->bnhcf', x, p)


def _ffn(x, w1, b1, w2, b2):
    return jax.nn.gelu(x @ w1.T + b1, approximate=False) @ w2.T + b2


def _forward(src, w):
    B, n, H, C = src.shape
    hd = C // NUM_HEADS
    qkv = (src @ w['qkv_w'].T + w['qkv_b']).reshape(B, n, H, 3, NUM_HEADS, hd)
    qkv = jnp.transpose(qkv, (3, 0, 1, 4, 2, 5))
    q, k, v = qkv[0], qkv[1], qkv[2]
    v_dp = _dyn_proj(v, w['dpv_w'], w['dpv_b'])
    k_dp = _dyn_proj(k, w['dpk_w'], w['dpk_b'])
    E = w['ema_matrix']
    s_tok = jnp.einsum('bnhed,bnhfd->bnhef', _ema(q, E), _ema(k_dp, E)) * (hd ** 0.5)
    o_tok = jnp.einsum('bnhef,bnhfd->bnhed', jax.nn.softmax(s_tok, -1), v_dp)
    s_hid = jnp.einsum('bnhae,bnhaf->bnhef', q, k) * (H ** 0.5)
    o_hid = jnp.einsum('bnhef,bnhaf->bnhae', jax.nn.softmax(s_hid, -1), v)
    o1 = _bn(o_tok.reshape(B, n, -1, C), w['bn1_g'], w['bn1_b'], w['bn1_m'], w['bn1_v'])
    o2 = _bn(o_hid.reshape(B, n, -1, C), w['bn2_g'], w['bn2_b'], w['bn2_m'], w['bn2_v'])
    src2 = _ffn(o1, w['ff1_w1'], w['ff1_b1'], w['ff1_w2'], w['ff1_b2']) \
         + _ffn(o2, w['ff2_w1'], w['ff2_b1'], w['ff2_w2'], w['ff2_b2'])
    out = src + src2
    return _bn(out, w['bn3_g'], w['bn3_b'], w['bn3_m'], w['bn3_v'])


_pforward = jax.pmap(_forward, axis_name='cores', in_axes=(0, None))


def kernel(**inputs) -> np.ndarray:
    src = np.asarray(inputs['src'], dtype=np.float32)
    B, n, H, C = src.shape
    w = {k: jnp.asarray(np.asarray(inputs[k], dtype=np.float32))
         for k in _WEIGHT_KEYS}
    # shard batch dim across the 8 cores
    src_sh = src.reshape(N_CORES, B // N_CORES, n, H, C)
    out_sh = _pforward(jnp.asarray(src_sh), w)
    out = np.asarray(out_sh).reshape(B, n, H, C)
    return out

